# revision 20
# baseline (speedup 1.0000x reference)
"""Spatial-reduction attention (PVT-style) on 8 TRN2 NeuronCores.

Strategy: pure data-parallel over batch (B=8 -> 1 batch element per core,
zero collectives). Per core, everything is computed in "feature-major"
(transposed) layout so that the attention-weight matrix E^T = exp(S^T)
lands with the context dim m on partitions -- exactly what the PV matmul
needs as its moving operand, so the big attention tensor is never
transposed on chip.

Key tricks:
  - conv(stride 2, 2x2) == patch-merge matmul; patches are gathered
    host-side, only for the m positions with mask!=0 (mask compression,
    1024 -> M_pad ~ 640), since masked context positions contribute
    nothing to the attention output.
  - mask + softmax denominator are folded into the PV matmul: the
    stationary operand V'' has 65 columns per head (64 = mask*V, 1 = mask),
    so row 64 of the PV output is the softmax denominator.
  - layernorm's ln_w/ln_b are folded into Wkv host-side; on-chip LN is a
    pure standardize using ones-matmul column stats + partition broadcast.
  - all matmuls run in bf16 (full PE rate).
  - scores PSUM is one 5-bank tile; exp per head is split [4 banks]+[1
    bank] so the next head's first 4 score matmuls only wait on the big
    exp read and the 5th decouples -> ACT (exp) paces the steady state at
    ~2.7us per head-slot with PE trailing just under it.
  - per head-slot the PE runs: scores mc0-3 (half-array paired), the
    previous head's PV chain, scores mc4, and one gap-filler chain --
    Q-proj for the next chunk (slots 0-3, per-head-pair so only the pair
    needed first blocks) or out-proj for the previous chunk (slots 4-7,
    after all divides landed).
  - prologue: inputs arrive as a few large DMAs (SWDGE issue is ~0.7us
    each); Q(0) fills the PE during the DMA wait (also HAM-warms it);
    Q(1) fills the LN-chain latency; a dummy exp preloads the ACT exp
    table before the steady loop.
"""

import math
import numpy as np

N_SEQ = 4096
DIM = 512
HEADS = 8
DH = 64
INNER = HEADS * DH
SR = 2
SCALE = DH ** -0.5
LN_EPS = 1e-5
B = 8
NCHUNK = 512          # n-tile size of the main loop
EH = DH + 1           # 65: V'' columns per head (64 V + 1 mask/denominator)


def _ensure_path():
    try:
        import concourse.bass  # noqa: F401
    except ImportError:
        import sys
        for p in ("/opt/trn_rl_repo", "/root/.axon_site/_ro/trn_rl_repo"):
            if p not in sys.path:
                sys.path.append(p)


def _m_pieces(m_pad):
    """Split [0, m_pad) into 128-aligned pieces of at most 512, so each
    piece covers whole m-chunks."""
    if m_pad <= 512:
        return [(0, m_pad)]
    nmc = m_pad // 128
    a = min(3, nmc - 1)
    return [(0, a * 128), (a * 128, m_pad)]


def _build(m_pad):
    _ensure_path()
    import concourse.bass as bass  # noqa: F401
    import concourse.mybir as mybir
    import concourse.tile as tile
    from concourse import bacc

    f32 = mybir.dt.float32
    bf16 = mybir.dt.bfloat16
    FT = mybir.ActivationFunctionType
    OP = mybir.AluOpType

    nmc = m_pad // 128
    assert nmc == 5, f"optimized build assumes nmc=5, got {nmc}"
    nA = nmc - 1           # m-chunks covered by the big exp instruction
    pieces = _m_pieces(m_pad)
    n_nc = N_SEQ // NCHUNK

    nc = bacc.Bacc()

    xt_e = nc.declare_dram_parameter("xt", [DIM, N_SEQ], bf16, isOutput=False)
    xp_e = nc.declare_dram_parameter("xp", [4 * DIM, m_pad], bf16, isOutput=False)
    w2_e = nc.declare_dram_parameter("w2", [4 * DIM, DIM], bf16, isOutput=False)
    wq_e = nc.declare_dram_parameter("wq", [128, 4, DIM], bf16, isOutput=False)
    wk_e = nc.declare_dram_parameter("wk", [128, 4, DIM], bf16, isOutput=False)
    wv_e = nc.declare_dram_parameter("wv", [128, 4, DIM], bf16, isOutput=False)
    wp_e = nc.declare_dram_parameter("wp", [128, 4, DIM], bf16, isOutput=False)
    convb_e = nc.declare_dram_parameter("convb", [128, 4], f32, isOutput=False)
    bk_e = nc.declare_dram_parameter("bk", [128, 4], f32, isOutput=False)
    bv_e = nc.declare_dram_parameter("bv", [DIM], f32, isOutput=False)
    bp_e = nc.declare_dram_parameter("bp", [128, 4], f32, isOutput=False)
    maskc_e = nc.declare_dram_parameter("maskc", [128, nmc], f32, isOutput=False)
    out_e = nc.declare_dram_parameter("out", [DIM, N_SEQ], f32, isOutput=True)

    def r(ap):
        return ap

    from contextlib import ExitStack

    with tile.TileContext(nc) as tc:
        with ExitStack() as stk:
            def pool(name, bufs, space="SBUF"):
                return stk.enter_context(
                    tc.tile_pool(name=name, bufs=bufs, space=space))

            wpool = pool("wts", 1)
            cpool = pool("consts", 1)
            xpcp = pool("stream", 1)
            w2p = pool("w2s", 1)
            ctxp = pool("ctx", 1)
            sqp = pool("sqs", 2)
            kvp = pool("kv", 1)
            xtp = pool("xtq", 3)
            qp = pool("qq", 2)
            ep = pool("ee", 2)
            opool = pool("oo", 3)
            yp = pool("yy", 3)
            smp = pool("small", 1)
            r1p = pool("r1p", 2)
            bcp = pool("bc", 2)
            rbp = pool("rbp", 2)
            s5pool = pool("ps_s5", 1, space="PSUM")   # 5 banks: scores/patch
            pvp = pool("ps_pv", 1, space="PSUM")      # 1 bank: PV out
            qpp = pool("ps_q", 1, space="PSUM")       # 1 bank: Q/K chains
            fpp = pool("ps_f", 1, space="PSUM")       # 1 bank: out/V chains

            # ---- constants; dummy exp at t=0 preloads the ACT exp table ----
            eps_sb = cpool.tile([1, 1], f32, tag="eps")
            nc.vector.memset(eps_sb[:], LN_EPS)
            warm_sb = cpool.tile([1, 1], f32, tag="warm")
            nc.scalar.activation(out=warm_sb[:], in_=eps_sb[:], func=FT.Exp)
            warmb_sb = cpool.tile([2, 1], f32, tag="warmb")
            nc.gpsimd.partition_broadcast(out_ap=warmb_sb[:], in_ap=eps_sb[:])

            wq_sb = wpool.tile([128, 4, DIM], bf16, tag="wq")
            nc.gpsimd.dma_start(out=wq_sb[:], in_=wq_e.ap())
            ones_sb = cpool.tile([128, 1], bf16, tag="ones")
            nc.vector.memset(ones_sb[:], 1.0)
            ones8_sb = cpool.tile([128, 8], f32, tag="ones8")
            nc.vector.memset(ones8_sb[:], 1.0)

            # ---- streaming input DMAs: few large transfers, piece-0 first ----
            xt_r = xt_e.ap().rearrange("(cc p) n -> p cc n", p=128)  # [128,4,N]
            xt_tiles = {}
            xt = xtp.tile([128, 4, NCHUNK], bf16, tag="xt")
            nc.sync.dma_start(out=xt[:], in_=xt_r[:, :, 0:NCHUNK])
            xt_tiles[0] = xt

            xp_r = xp_e.ap().rearrange("(kc p) m -> kc p m", p=128)   # [16,128,m]
            w2_r = w2_e.ap().rearrange("(g kc p) co -> g p kc co", g=2, p=128)
            xp_t = []
            for kc in range(16):
                xpk = xpcp.tile([128, m_pad], bf16, tag=f"xp{kc}")
                xp_t.append(xpk)
            w2_t0 = w2p.tile([128, 8, DIM], bf16, tag="w20")
            w2_t1 = w2p.tile([128, 8, DIM], bf16, tag="w21")
            w2_t = [w2_t0, w2_t1]
            # big streams all on the sync queue, in consumption order
            nc.sync.dma_start(out=w2_t[0][:], in_=w2_r[0])
            for kc in range(8):
                nc.sync.dma_start(out=xp_t[kc][:], in_=xp_r[kc, :, :])
            nc.sync.dma_start(out=w2_t[1][:], in_=w2_r[1])
            for kc in range(8, 16):
                nc.sync.dma_start(out=xp_t[kc][:], in_=xp_r[kc, :, :])
            xt1 = xtp.tile([128, 4, NCHUNK], bf16, tag="xt")
            nc.sync.dma_start(out=xt1[:], in_=xt_r[:, :, NCHUNK:2 * NCHUNK])
            xt_tiles[1] = xt1
            # small weights/consts on the gpsimd queue
            wk_sb = wpool.tile([128, 4, DIM], bf16, tag="wk")
            nc.gpsimd.dma_start(out=wk_sb[:], in_=wk_e.ap())
            wv_sb = wpool.tile([128, 4, DIM], bf16, tag="wv")
            nc.gpsimd.dma_start(out=wv_sb[:], in_=wv_e.ap())
            wp_sb = wpool.tile([128, 4, DIM], bf16, tag="wp")
            nc.gpsimd.dma_start(out=wp_sb[:], in_=wp_e.ap())
            convb_sb = cpool.tile([128, 4], f32, tag="convb")
            nc.gpsimd.dma_start(out=convb_sb[:], in_=convb_e.ap())
            bk_sb = cpool.tile([128, 4], f32, tag="bk")
            nc.gpsimd.dma_start(out=bk_sb[:], in_=bk_e.ap())
            bp_sb = cpool.tile([128, 4], f32, tag="bp")
            nc.gpsimd.dma_start(out=bp_sb[:], in_=bp_e.ap())
            maskc_sb = cpool.tile([128, nmc], f32, tag="maskc")
            nc.gpsimd.dma_start(out=maskc_sb[:], in_=maskc_e.ap())
            bv_bc = cpool.tile([128, DIM], f32, tag="bvbc")
            nc.gpsimd.dma_start(out=bv_bc[:], in_=bv_e.ap().partition_broadcast(128))
            xt2 = xtp.tile([128, 4, NCHUNK], bf16, tag="xt")
            nc.sync.dma_start(out=xt2[:], in_=xt_r[:, :, 2 * NCHUNK:3 * NCHUNK])
            xt_tiles[2] = xt2

            def w2kc(kc):
                return w2_t[kc // 8][:, kc % 8, :]

            def xpkc(kc, p0, p1):
                return xp_t[kc][:, p0:p1]

            # ---- persistent PSUM scores tiles: 4-bank A + 1-bank D ----
            # (separate tiles so Tile's tile-granular dep tracking lets the
            # next head's mc4 score matmul bypass the big eA read)
            sA4 = s5pool.tile([128, nA, NCHUNK], f32, tag="sA4")
            sDt = s5pool.tile([128, NCHUNK], f32, tag="sDt")

            # ---- Q projection helpers ----
            def q_chain(xt_sb, q_sb, ic, psum_pool=None, psum_tag="q"):
                pool_ = psum_pool or qpp
                ps = pool_.tile([128, NCHUNK], f32, tag=psum_tag)
                for cc in range(4):
                    nc.tensor.matmul(
                        ps[:],
                        lhsT=r(wq_sb[:, cc, ic * 128:(ic + 1) * 128]),
                        rhs=r(xt_sb[:, cc, :]),
                        start=(cc == 0), stop=(cc == 3),
                    )
                nc.vector.tensor_copy(out=q_sb[:, ic, :], in_=ps[:])

            def q_swap_dma(q_sb, q_sw):
                # partition-shift copies via DMA (slow path on DVE)
                nc.sync.dma_start(out=q_sw[0:64, :, :], in_=q_sb[64:128, :, :])
                nc.sync.dma_start(out=q_sw[64:128, :, :], in_=q_sb[0:64, :, :])

            # ---- HAM warm-up: ~3.5us of tiny dependency-free matmuls so
            # Q(0) and the patch run at 2.4 GHz from the start ----
            warm_ps = pvp.tile([8, 64], f32, tag="pv")
            for _ in range(52):
                nc.tensor.matmul(warm_ps[0:8, 0:8], lhsT=r(ones8_sb[0:1, :]),
                                 rhs=r(ones8_sb[0:1, :]),
                                 start=True, stop=True)

            # ---- prologue Q for chunk 0 (PE work during DMA wait);
            # alternate PSUM banks so the q-copy WAR doesn't serialize ----
            q_tiles = {}
            q_sb0 = qp.tile([128, 4, NCHUNK], bf16, tag="q")
            q_sw0 = qp.tile([128, 4, NCHUNK], bf16, tag="qsw")
            q_tiles[0] = (q_sb0, q_sw0)
            for ic in range(4):
                q_chain(xt_tiles[0], q_tiles[0][0], ic,
                        psum_pool=(qpp if ic % 2 == 0 else fpp),
                        psum_tag=("q" if ic % 2 == 0 else "fin"))
            q_swap_dma(*q_tiles[0])

            # ---- phase 1: patch-merge ctx^T (feature-major) + layernorm ----
            ctx_raw = ctxp.tile([128, 4, m_pad], bf16, tag="craw")
            ctxn = ctxp.tile([128, 4, m_pad], bf16, tag="cn")
            k_sb = kvp.tile([128, 4, m_pad], bf16, tag="k")
            k_sw = kvp.tile([128, 4, m_pad], bf16, tag="ksw")
            v2_sb = kvp.tile([128, nmc, HEADS * EH], bf16, tag="v2")
            bv3 = bv_bc[:].rearrange("p (h d) -> p h d", d=DH)

            def patch_mm(p0, p1):
                """Patch-merge accumulation matmuls for one piece (PE)."""
                pw = p1 - p0
                for ki, kc in enumerate(range(16)):
                    for cco in range(4):
                        nc.tensor.matmul(
                            sA4[:, cco, :pw],
                            lhsT=r(w2kc(kc)[:, cco * 128:(cco + 1) * 128]),
                            rhs=r(xpkc(kc, p0, p1)),
                            start=(ki == 0), stop=(ki == 15),
                        )

            def bias_sq(p0, p1):
                """Conv-bias add + squares (DVE)."""
                pw = p1 - p0
                sqs = []
                for cco in range(4):
                    nc.vector.tensor_scalar(
                        out=ctx_raw[:, cco, p0:p1], in0=sA4[:, cco, :pw],
                        scalar1=convb_sb[:, cco:cco + 1], scalar2=None,
                        op0=OP.add,
                    )
                    sq_s = sqp.tile([128, NCHUNK], bf16, tag="sqs")
                    nc.vector.tensor_tensor(
                        out=sq_s[:, :pw], in0=ctx_raw[:, cco, p0:p1],
                        in1=ctx_raw[:, cco, p0:p1], op=OP.mult,
                    )
                    sqs.append(sq_s)
                return sqs

            def stats_mm(p0, p1, sqs):
                """Column-stat ones-matmuls (PE)."""
                pw = p1 - p0
                mu_ps = pvp.tile([EH, NCHUNK], f32, tag="pv")
                ss_ps = fpp.tile([128, NCHUNK], f32, tag="fin")
                for cc in range(4):
                    nc.tensor.matmul(
                        mu_ps[0:1, :pw], lhsT=r(ones_sb[:]),
                        rhs=r(ctx_raw[:, cc, p0:p1]),
                        start=(cc == 0), stop=(cc == 3),
                    )
                    nc.tensor.matmul(
                        ss_ps[0:1, :pw], lhsT=r(ones_sb[:]),
                        rhs=r(sqs[cc][:, :pw]),
                        start=(cc == 0), stop=(cc == 3),
                    )
                return mu_ps, ss_ps

            def ln_chain(p0, p1, mu_ps, ss_ps):
                """LN scalar chain + broadcasts + apply (DVE/ACT/gpsimd)."""
                pw = p1 - p0
                m1n = smp.tile([1, NCHUNK], f32, tag="m1n")
                nc.vector.tensor_scalar(
                    out=m1n[:, :pw], in0=mu_ps[0:1, :pw],
                    scalar1=-1.0 / DIM, scalar2=None, op0=OP.mult,
                )
                v1 = smp.tile([1, NCHUNK], f32, tag="v1")
                nc.vector.tensor_scalar(
                    out=v1[:, :pw], in0=ss_ps[0:1, :pw],
                    scalar1=1.0 / DIM, scalar2=None, op0=OP.mult,
                )
                m2 = smp.tile([1, NCHUNK], f32, tag="m2")
                nc.vector.tensor_tensor(
                    out=m2[:, :pw], in0=m1n[:, :pw], in1=m1n[:, :pw], op=OP.mult
                )
                var = smp.tile([1, NCHUNK], f32, tag="var")
                nc.vector.tensor_tensor(
                    out=var[:, :pw], in0=v1[:, :pw], in1=m2[:, :pw],
                    op=OP.subtract,
                )
                std = smp.tile([1, NCHUNK], f32, tag="std")
                nc.scalar.activation(
                    out=std[:, :pw], in_=var[:, :pw], func=FT.Sqrt,
                    bias=eps_sb[:],
                )
                # reciprocal via partition-transpose (DVE recip on [1, pw] is
                # ~7ns/elem; on [128, pw/128] it is ~free)
                npc = pw // 128
                stdT = smp.tile([128, 4], f32, tag="stdT")
                nc.gpsimd.dma_start(
                    out=stdT[:, :npc].rearrange("p c -> p (c)"),
                    in_=std[:, :pw])
                rstdT = smp.tile([128, 4], f32, tag="rstdT")
                nc.vector.reciprocal(out=rstdT[:, :npc], in_=stdT[:, :npc])
                rstd = smp.tile([1, NCHUNK], f32, tag="rstd")
                nc.gpsimd.dma_start(
                    out=rstd[:, :pw],
                    in_=rstdT[:, :npc].rearrange("p c -> p (c)"))
                tsh = smp.tile([1, NCHUNK], f32, tag="tsh")
                nc.vector.tensor_tensor(
                    out=tsh[:, :pw], in0=m1n[:, :pw], in1=rstd[:, :pw],
                    op=OP.mult,
                )
                r_bc = bcp.tile([128, NCHUNK], f32, tag="rbc")
                nc.gpsimd.partition_broadcast(out_ap=r_bc[:, :pw],
                                              in_ap=rstd[:, :pw])
                t_bc = bcp.tile([128, NCHUNK], f32, tag="tbc")
                nc.gpsimd.partition_broadcast(out_ap=t_bc[:, :pw],
                                              in_ap=tsh[:, :pw])
                for cc in range(4):
                    nc.vector.tensor_tensor(
                        out=ctxn[:, cc, p0:p1], in0=ctx_raw[:, cc, p0:p1],
                        in1=r_bc[:, :pw], op=OP.mult,
                    )
                    nc.vector.tensor_tensor(
                        out=ctxn[:, cc, p0:p1], in0=ctxn[:, cc, p0:p1],
                        in1=t_bc[:, :pw], op=OP.add,
                    )

            def k_piece(p0, p1):
                pw = p1 - p0
                for kc in range(4):
                    ps = qpp.tile([128, NCHUNK], f32, tag="q")
                    for cc in range(4):
                        nc.tensor.matmul(
                            ps[:, :pw],
                            lhsT=r(wk_sb[:, cc, kc * 128:(kc + 1) * 128]),
                            rhs=r(ctxn[:, cc, p0:p1]),
                            start=(cc == 0), stop=(cc == 3),
                        )
                    nc.vector.tensor_scalar(
                        out=k_sb[:, kc, p0:p1], in0=ps[:, :pw],
                        scalar1=bk_sb[:, kc:kc + 1], scalar2=None, op0=OP.add,
                    )
                nc.vector.tensor_copy(
                    out=k_sw[0:64, :, p0:p1], in_=k_sb[64:128, :, p0:p1])
                nc.vector.tensor_copy(
                    out=k_sw[64:128, :, p0:p1], in_=k_sb[0:64, :, p0:p1])

            def v_piece(p0, p1):
                for mc in range(p0 // 128, p1 // 128):
                    ps = fpp.tile([128, NCHUNK], f32, tag="fin")
                    for cc in range(4):
                        nc.tensor.matmul(
                            ps[:],
                            lhsT=r(ctxn[:, cc, mc * 128:(mc + 1) * 128]),
                            rhs=r(wv_sb[:, cc, :]),
                            start=(cc == 0), stop=(cc == 3),
                        )
                    v3 = v2_sb[:, mc, :].rearrange("p (h e) -> p h e", e=EH)
                    nc.vector.tensor_tensor(
                        out=v3[:, :, 0:DH],
                        in0=ps[:].rearrange("p (h d) -> p h d", d=DH),
                        in1=bv3, op=OP.add,
                    )
                    nc.vector.tensor_scalar(
                        out=v3[:, :, 0:DH], in0=v3[:, :, 0:DH],
                        scalar1=maskc_sb[:, mc:mc + 1], scalar2=None,
                        op0=OP.mult,
                    )
                    nc.vector.tensor_scalar(
                        out=v3[:, :, DH:EH],
                        in0=ones8_sb[:].rearrange("p (h u) -> p h u", u=1),
                        scalar1=maskc_sb[:, mc:mc + 1], scalar2=None,
                        op0=OP.mult,
                    )

            (pa, pb) = pieces
            patch_mm(*pa)
            sqs_a = bias_sq(*pa)
            # Q(1) fills the PE during piece-a's bias/sq DVE latency
            q1_sb = qp.tile([128, 4, NCHUNK], bf16, tag="q")
            q1_sw = qp.tile([128, 4, NCHUNK], bf16, tag="qsw")
            q_tiles[1] = (q1_sb, q1_sw)
            for ic in range(4):
                q_chain(xt_tiles[1], q_tiles[1][0], ic,
                        psum_pool=(qpp if ic % 2 == 0 else fpp),
                        psum_tag=("q" if ic % 2 == 0 else "fin"))
            q_swap_dma(*q_tiles[1])
            st_a = stats_mm(*pa, sqs_a)
            ln_chain(*pa, *st_a)
            patch_mm(*pb)
            sqs_b = bias_sq(*pb)
            k_piece(*pa)
            st_b = stats_mm(*pb, sqs_b)
            ln_chain(*pb, *st_b)
            # re-warm the exp table set (Sqrt above switched sets) while the
            # PE chews on K/V
            warm2_sb = cpool.tile([1, 1], f32, tag="warm2")
            nc.scalar.activation(out=warm2_sb[:], in_=eps_sb[:], func=FT.Exp)
            v_piece(*pa)
            k_piece(*pb)
            v_piece(*pb)

            # ---- phase 3: main n-chunk loop, head-slot pipeline ----
            def scores_mm(q_sb, q_sw, h, mc):
                hc = h // 2
                half = mc % 2
                if (h % 2) == half:
                    ksrc, qsrc = k_sb, q_sb
                else:
                    ksrc, qsrc = k_sw, q_sw
                hp = half * 64
                dst = sA4[:, mc, :] if mc < nA else sDt[:]
                nc.tensor.matmul(
                    dst,
                    lhsT=r(ksrc[hp:hp + 64, hc, mc * 128:(mc + 1) * 128]),
                    rhs=r(qsrc[hp:hp + 64, hc, :]),
                    start=True, stop=True,
                )

            def pv_drain(h, eA, eD, o_st):
                pv = pvp.tile([EH, NCHUNK], f32, tag="pv")
                for mc in range(nmc):
                    src = eA[:, mc, :] if mc < nA else eD[:]
                    nc.tensor.matmul(
                        pv[:],
                        lhsT=r(v2_sb[:, mc, h * EH:(h + 1) * EH]),
                        rhs=r(src),
                        start=(mc == 0), stop=(mc == nmc - 1),
                    )
                nc.vector.tensor_copy(out=o_st[:, h, :], in_=pv[:])

            # divide2 is pipelined over 3 head-slots so none of its DVE ops
            # ever sits unready at the head of the DVE FIFO:
            #   stage1 (slot s):   denominator gather DMA
            #   stage2 (slot s+1): reciprocal + scatter DMA + broadcasts
            #   stage3 (slot s+2): the two o_sb multiplies
            def div_stage1(g, o_st):
                dT = r1p.tile([128, 2 * NCHUNK // 128], bf16, tag="dT")
                nc.gpsimd.dma_start(
                    out=dT[:],
                    in_=o_st[DH:EH, 2 * g:2 * g + 2, :].rearrange(
                        "p a b -> p (a b)"))
                return dT

            def div_stage2(g, dT):
                rT = r1p.tile([128, 2 * NCHUNK // 128], bf16, tag="rT")
                with nc.allow_low_precision("bf16 softmax denoms"):
                    nc.vector.reciprocal(out=rT[:], in_=dT[:])
                rfl = r1p.tile([1, 2, NCHUNK], bf16, tag="rf")
                nc.gpsimd.dma_start(
                    out=rfl[:].rearrange("p a b -> p (a b)"),
                    in_=rT[:])
                return rfl

            def div_stage2b(g, rfl):
                rbs = []
                for j in (0, 1):
                    rb = rbp.tile([64, NCHUNK], bf16, tag="rb")
                    nc.gpsimd.partition_broadcast(
                        out_ap=rb[:], in_ap=rfl[0:1, j, :])
                    rbs.append(rb)
                return rbs

            def div_stage3(g, rbs, o_st, o_sb):
                for j, hh in enumerate((2 * g, 2 * g + 1)):
                    nc.vector.tensor_tensor(
                        out=o_sb[(hh % 2) * 64:(hh % 2) * 64 + 64,
                                 hh // 2, :],
                        in0=o_st[0:DH, hh, :], in1=rbs[j],
                        op=OP.mult,
                    )

            def out_chain(o_sb, cc, n0):
                ps = fpp.tile([128, NCHUNK], f32, tag="fin")
                for ic in range(4):
                    nc.tensor.matmul(
                        ps[:],
                        lhsT=r(wp_sb[:, ic, cc * 128:(cc + 1) * 128]),
                        rhs=r(o_sb[:, ic, :]),
                        start=(ic == 0), stop=(ic == 3),
                    )
                y_sb = yp.tile([128, NCHUNK], f32, tag="y")
                nc.vector.tensor_scalar(
                    out=y_sb[:], in0=ps[:], scalar1=bp_sb[:, cc:cc + 1],
                    scalar2=None, op0=OP.add,
                )
                nc.sync.dma_start(
                    out=out_e.ap()[cc * 128:(cc + 1) * 128, n0:n0 + NCHUNK],
                    in_=y_sb[:],
                )

            prev = None       # (h, eA, eD, o_st, o_sb)
            prev_chunk = None  # (o_sb, n0) of previous chunk, for out-proj
            cc3_src = None     # (o_sb, n0) two chunks back, for out-proj cc3
            pending = []      # [(due_slot, fn)] deferred divide stages
            slot = 0

            def flush(s):
                ready = [p for p in pending if p[0] <= s]
                pending[:] = [p for p in pending if p[0] > s]
                for _, fn in ready:
                    fn()

            def sched_divide(g, d_ost, d_osb, s):
                # pipeline: dT now; recip+rfl @s+2; broadcasts @s+3; TT @s+5
                dT = div_stage1(g, d_ost)

                def mk2(g=g, dT=dT, d_ost=d_ost, d_osb=d_osb, s=s):
                    rfl = div_stage2(g, dT)

                    def mk2b():
                        rbs = div_stage2b(g, rfl)
                        pending.append(
                            (s + 5,
                             lambda: div_stage3(g, rbs, d_ost, d_osb)))
                    pending.append((s + 3, mk2b))
                pending.append((s + 2, mk2))

            for ni in range(n_nc):
                n0 = ni * NCHUNK
                q_sb, q_sw = q_tiles[ni]
                o_sb = opool.tile([128, 4, NCHUNK], bf16, tag="o")
                o_st = opool.tile([EH, 8, NCHUNK], bf16, tag="ost")
                if 1 <= ni <= 6:
                    qn_sb = qp.tile([128, 4, NCHUNK], bf16, tag="q")
                    qn_sw = qp.tile([128, 4, NCHUNK], bf16, tag="qsw")
                    q_tiles[ni + 1] = (qn_sb, qn_sw)
                for h in range(HEADS):
                    flush(slot)
                    # scores mc 0..3 -> banks 0..3 (wait previous eA read)
                    for mc in range(nA):
                        scores_mm(q_sb, q_sw, h, mc)
                    eA = ep.tile([128, nA, NCHUNK], bf16, tag="eA")
                    eD = ep.tile([128, NCHUNK], bf16, tag="eD")
                    nc.scalar.activation(
                        out=eA[:], in_=sA4[:], func=FT.Exp)
                    # trailing PV for the previous head-slot (head 7's PV was
                    # already drained inside its own slot)
                    if prev is not None and prev[0] != 7:
                        ph, peA, peD, po_st, po_sb = prev
                        pv_drain(ph, peA, peD, po_st)
                    # scores mc 4 -> bank 4 (waits previous eD read only)
                    scores_mm(q_sb, q_sw, h, nA)
                    nc.scalar.activation(
                        out=eD[:], in_=sDt[:], func=FT.Exp)
                    # gap-filler chains: Q-proj (next chunk) in slots 0-3,
                    # out-proj cc0-2 (previous chunk) in slots 4-6; cc3 of
                    # the chunk before that lands in slot 0
                    if h == 0 and cc3_src is not None:
                        out_chain(cc3_src[0], 3, cc3_src[1])
                    if h < 4:
                        if 1 <= ni <= 6:
                            q_chain(xt_tiles[ni + 1], q_tiles[ni + 1][0], h)
                            if h == 3:
                                q_swap_dma(*q_tiles[ni + 1])
                    elif h < 7:
                        if prev_chunk is not None:
                            out_chain(prev_chunk[0], h - 4, prev_chunk[1])
                        if h == 6 and 1 <= ni <= 5:
                            xt = xtp.tile([128, 4, NCHUNK], bf16, tag="xt")
                            nc.sync.dma_start(
                                out=xt[:],
                                in_=xt_r[:, :,
                                         (ni + 2) * NCHUNK:(ni + 3) * NCHUNK])
                            xt_tiles[ni + 2] = xt
                    # divides trail (gpsimd/DVE work pipelined over later slots)
                    if prev is not None and prev[0] in (1, 3, 5):
                        sched_divide(prev[0] // 2, prev[3], prev[4], slot)
                    if h == 7:
                        # head 7's PV drains in its own slot so its divide
                        # starts a slot early and out-proj never waits on it
                        pv_drain(7, eA, eD, o_st)
                        sched_divide(3, o_st, o_sb, slot)
                    prev = (h, eA, eD, o_st, o_sb)
                    slot += 1
                cc3_src = prev_chunk
                prev_chunk = (o_sb, n0)

            # ---- tail: flush remaining divides, then final out-projs ----
            while pending:
                flush(slot + 10)
                slot += 10
            if cc3_src is not None:
                out_chain(cc3_src[0], 3, cc3_src[1])
            for cc in range(4):
                out_chain(prev_chunk[0], cc, prev_chunk[1])

    nc.finalize()
    return nc


def _prep_inputs(x, mask, Wq, Wkv, conv_w, conv_b, ln_w, ln_b, Wp, bp, W):
    """Host-side sharding + layout prep. Returns (in_maps, m_pad)."""
    import ml_dtypes
    bf16 = ml_dtypes.bfloat16
    x = np.ascontiguousarray(np.asarray(x, dtype=np.float32))
    mask = np.asarray(mask, dtype=np.float32)
    Wq = np.asarray(Wq, dtype=np.float32)
    Wkv = np.asarray(Wkv, dtype=np.float32)
    conv_w = np.asarray(conv_w, dtype=np.float32)
    conv_b = np.asarray(conv_b, dtype=np.float32)
    ln_w = np.asarray(ln_w, dtype=np.float32)
    ln_b = np.asarray(ln_b, dtype=np.float32)
    Wp = np.asarray(Wp, dtype=np.float32)
    bp = np.asarray(bp, dtype=np.float32)

    Wm = W // SR
    kb = [int((mask[b] != 0).sum()) for b in range(B)]
    m_pad = max(256, ((max(kb) + 127) // 128) * 128)

    def rearr_w(w):  # [512, 512] -> [128, 4, 512] with [p, cc, :] = w[cc*128+p]
        return np.ascontiguousarray(w.reshape(4, 128, -1).transpose(1, 0, 2))

    def rearr_b(v):  # [512] -> [128, 4]
        return np.ascontiguousarray(v.reshape(4, 128).T)

    w2 = np.ascontiguousarray(
        conv_w.transpose(2, 3, 1, 0).reshape(4 * DIM, DIM)).astype(bf16)
    wq_in = rearr_w(Wq.T * np.float32(SCALE)).astype(bf16)
    wk_in = rearr_w((Wkv[:INNER] * ln_w).T).astype(bf16)
    wv_in = rearr_w((Wkv[INNER:] * ln_w).T).astype(bf16)
    wp_in = rearr_w(Wp.T).astype(bf16)
    bk_in = rearr_b(Wkv[:INNER] @ ln_b)
    bv_in = np.ascontiguousarray(Wkv[INNER:] @ ln_b)
    convb_in = rearr_b(conv_b)
    bp_in = rearr_b(bp)

    in_maps = []
    for b in range(B):
        xb = x[b]
        sel = np.nonzero(mask[b] != 0)[0]
        sel_pad = np.zeros(m_pad, dtype=np.int64)
        sel_pad[: len(sel)] = sel
        i = sel_pad // Wm
        j = sel_pad % Wm
        n_idx = np.stack(
            [(2 * i + di) * W + (2 * j + dj) for di in (0, 1) for dj in (0, 1)]
        )  # [4, m_pad], p = di*2+dj
        xp = xb[n_idx]  # [4, m_pad, 512]
        xp = np.ascontiguousarray(
            xp.transpose(0, 2, 1).reshape(4 * DIM, m_pad))
        maskc = (np.arange(m_pad) < len(sel)).astype(np.float32)
        maskc_in = np.ascontiguousarray(maskc.reshape(-1, 128).T)
        in_maps.append({
            "xt": np.ascontiguousarray(xb.T).astype(bf16),
            "xp": xp.astype(bf16),
            "w2": w2,
            "wq": wq_in,
            "wk": wk_in,
            "wv": wv_in,
            "wp": wp_in,
            "convb": convb_in,
            "bk": bk_in,
            "bv": bv_in,
            "bp": bp_in,
            "maskc": maskc_in,
        })
    return in_maps, m_pad


_BUILD_CACHE = {}


def kernel(x, H, W, mask, Wq, Wkv, conv_w, conv_b, ln_w, ln_b, Wp, bp,
           _results_hook=None):
    H = int(H)
    W = int(W)
    assert (H, W) == (64, 64) and x.shape == (B, N_SEQ, DIM), (H, W, x.shape)

    in_maps, m_pad = _prep_inputs(
        x, mask, Wq, Wkv, conv_w, conv_b, ln_w, ln_b, Wp, bp, W)

    if m_pad not in _BUILD_CACHE:
        _BUILD_CACHE[m_pad] = _build(m_pad)
    nc = _BUILD_CACHE[m_pad]

    _ensure_path()
    from concourse.bass_utils import run_bass_kernel_spmd

    res = run_bass_kernel_spmd(nc, in_maps, core_ids=list(range(B)))
    if _results_hook is not None:
        _results_hook(res)

    out = np.empty((B, N_SEQ, DIM), dtype=np.float32)
    for b in range(B):
        out[b] = res.results[b]["out"].T
    return out
